# revision 16
# baseline (speedup 1.0000x reference)
"""Trainium2 Bass kernel for nn_AU_Net_3573412790684 (GNN message passing).

Strategy (8 NeuronCores, SPMD + collectives):
  - Node dim padded 1026 -> NP=1152 (9*128); nodes sharded 144/core.
  - Activations feature-major (X^T layout); weight-column tensor-parallel
    GEMMs with AllGather of activation slices between layers.
  - GDC exact PPR via Neumann doubling on G = M^T (row-sharded);
    top-128 per S-column via DVE max8/match_replace; column normalize.
  - GCN layers as dense matmuls vs host-built AhatT; their node-major lhsT
    operands produced by PE tile transposes of feature-major results.
  - All matmul operands float32r (fp32 storage, fast PE mode at N>=256).
  - Fused passes: one zT stream serves dr_w + ec2[:4096] + g1_w; one z1T
    stream serves ec2[4096:6144] + g2_w.

Per-core 144-row state in [128, 2*NP] block tiles (block1 = rows 128..143 in
partitions 0..15).  PSUM tags: pA(bufs2) pB pC pD pE(bufs1) tr(bufs2) = 8 banks.
"""
import sys
import numpy as np

sys.path.insert(0, "/opt/trn_rl_repo")
import concourse.bass as bass
from concourse import bacc
import concourse.mybir as mybir
import concourse.tile as tile
from concourse import bass_utils

from trnutil import legalize_matmul_waits

F32 = mybir.dt.float32
F32R = mybir.dt.float32r
AF = mybir.ActivationFunctionType

N = 1026
NP = 1152
S = NP // 8
DX = 4096
INS = 8192
JH = 2048
H0 = 4096
H1 = 2048
H2 = 1024
OUTS = 512
NL = 10
TOPK = 128
NSQ = 8
NCORES = 8
NC3 = [(0, 384), (384, 384), (768, 384)]     # full width (GDC chain)
NCF = [(0, 384), (384, 384), (768, 258)]     # feature gemms: skip pad cols
BLKS = [(0, 0, 128), (1, 128, 16)]

PS_TAGS = ["pA", "pA", "pB", "pC"]           # tp_gemm m-tile psum tags
PS_BUFS = [2, 2, 1, 1]


def _ceil(a, b):
    return -(-a // b)


def _mtiles(M):
    out, o = [], 0
    while o < M:
        t = min(128, M - o)
        out.append((o, t))
        o += t
    return out


class Prog:
    def __init__(self):
        self.nc = bacc.Bacc("TRN2", target_bir_lowering=False, debug=False,
                            num_devices=NCORES)
        self.uid = 0

    def name(self, p):
        self.uid += 1
        return f"{p}_{self.uid}"


def bv(t, bi, n_off=0, n_sz=NP, rows=None):
    r = (128 if bi == 0 else 16) if rows is None else rows
    return t[0:r, bi * NP + n_off: bi * NP + n_off + n_sz]


def tp_gemm(P, sb, ps, kxm_srcs, kxn_srcs, M, epilogue, n_chunks=NCF,
            cache_kxm=True, carry_in=None, carry_out=False):
    """out[M, n] += kxm^T @ kxn.  carry_in/carry_out: split-K across calls."""
    nc = P.nc
    ktiles = []
    for si, (ap, rows) in enumerate(kxm_srcs):
        for r in range(0, rows, 128):
            ktiles.append((si, r))
    nkt = len(ktiles)
    rh = []
    for si, (ap, rows) in enumerate(kxn_srcs):
        for r in range(0, rows, 128):
            rh.append((si, r))
    assert len(rh) == nkt
    mts = _mtiles(M)

    kxm_sb = None
    if cache_kxm:
        kxm_sb = sb.tile([128, nkt * M], F32R, name=P.name("kxmC"), tag="kxmC")
        for kt, (si, r) in enumerate(ktiles):
            nc.sync.dma_start(kxm_sb[:, kt * M:(kt + 1) * M],
                              kxm_srcs[si][0][r:r + 128, :])

    psums_all = carry_in if carry_in is not None else {}
    for ci, (n_off, n_sz) in enumerate(n_chunks):
        if carry_in is not None:
            psums = psums_all[ci]
        else:
            psums = [ps.tile([m_sz, n_sz], F32, name=P.name("psg"),
                             tag=PS_TAGS[mi], bufs=PS_BUFS[mi])
                     for mi, (m_off, m_sz) in enumerate(mts)]
            psums_all[ci] = psums
        for kt in range(nkt):
            si, r = rh[kt]
            rt = sb.tile([128, n_sz], F32R, name=P.name("rhs"), tag="rhs", bufs=4)
            nc.sync.dma_start(rt[:], kxn_srcs[si][0][r:r + 128, n_off:n_off + n_sz])
            for mi, (m_off, m_sz) in enumerate(mts):
                if cache_kxm:
                    lh = kxm_sb[:, kt * M + m_off: kt * M + m_off + m_sz]
                else:
                    lht = sb.tile([128, m_sz], F32R, name=P.name("lhs"),
                                  tag="lhs", bufs=4)
                    nc.sync.dma_start(lht[:], kxm_srcs[ktiles[kt][0]][0][
                        ktiles[kt][1]:ktiles[kt][1] + 128, m_off:m_off + m_sz])
                    lh = lht[:]
                nc.tensor.matmul(psums[mi][:], lh, rt[:],
                                 start=(kt == 0 and carry_in is None),
                                 stop=(kt == nkt - 1 and not carry_out))
        if not carry_out:
            for mi, (m_off, m_sz) in enumerate(mts):
                epilogue(mi, m_off, m_sz, n_off, n_sz, psums[mi])
    return psums_all


def act_epilogue(P, sb, out_dram, bias_tile, func, out_sb_fn=None):
    nc = P.nc

    def ep(mi, m_off, m_sz, n_off, n_sz, psum):
        t = sb.tile([m_sz, n_sz], F32R, name=P.name("ep"), tag="ep", bufs=3)
        if bias_tile is not None and func == AF.Copy:
            nc.vector.tensor_scalar_add(t[:], psum[:], bias_tile[0:m_sz, mi:mi + 1])
        elif bias_tile is not None:
            nc.scalar.activation(t[:], psum[:], func,
                                 bias=bias_tile[0:m_sz, mi:mi + 1])
        else:
            nc.scalar.activation(t[:], psum[:], func)
        if out_dram is not None:
            nc.sync.dma_start(out_dram[m_off:m_off + m_sz, n_off:n_off + n_sz], t[:])
        if out_sb_fn is not None:
            nc.vector.tensor_copy(out_sb_fn(mi, m_off, m_sz, n_off, n_sz), t[:])
    return ep


def load_bias(P, sb, bias_dram, M):
    nc = P.nc
    t = sb.tile([128, _ceil(M, 128)], F32, name=P.name("bias"),
                tag=P.name("bias"), bufs=1)
    for mi, (m_off, m_sz) in enumerate(_mtiles(M)):
        nc.sync.dma_start(t[:m_sz, mi:mi + 1], bias_dram[m_off:m_off + m_sz, :])
    return t


def allgather(P, dr, slice_dram, full_shape, name):
    nc = P.nc
    full = dr.tile(full_shape, F32R, name=name, addr_space="Shared")
    nc.gpsimd.collective_compute(
        "AllGather", mybir.AluOpType.bypass,
        replica_groups=[list(range(NCORES))],
        ins=[slice_dram.opt()], outs=[full.opt()])
    return full


def build_program():
    P = Prog()
    nc = P.nc

    def inp(name, shape, dt=F32R):
        return nc.dram_tensor(name, shape, dt, kind="ExternalInput")

    xgT = inp("xgT", [INS, NP])
    xcol = inp("xcol", [NP, DX // 8])
    ahatT = inp("ahatT", [NP, NP])
    eyeT = inp("eyeT", [S, NP])
    vmask = inp("vmask", [1, NP], F32)
    w_jw1 = inp("w_jw1", [INS, JH // 8]); b_jb1 = inp("b_jb1", [JH // 8, 1], F32)
    w_jw2 = inp("w_jw2", [JH, S]); b_jb2 = inp("b_jb2", [S, 1], F32)
    w_ec1x = inp("w_ec1x", [DX, H0 // 8])
    w_ec1g = inp("w_ec1g", [DX, H0 // 8]); b_ec1 = inp("b_ec1", [H0 // 8, 1], F32)
    w_zp = inp("w_zp", [H0, 512])            # [dr_w | ec2_w[:DX] | g1_w]
    b_dr = inp("b_dr", [H2 // 8, 1], F32)
    w_g1gx = inp("w_g1gx", [DX, H1 // 8])
    b_g1 = inp("b_g1", [H1 // 8, 1], F32)
    w_z1p = inp("w_z1p", [H1, 256])          # [ec2_w[DX:DX+H1] | g2_w]
    b_g2 = inp("b_g2", [H2 // 8, 1], F32)
    w_ec2c = inp("w_ec2c", [H2, H2 // 8]); b_ec2 = inp("b_ec2", [H2 // 8, 1], F32)
    w_ec3 = inp("w_ec3", [H2, OUTS // 8]); b_ec3 = inp("b_ec3", [OUTS // 8, 1], F32)
    w_out = inp("w_out", [OUTS, NL]); b_out = inp("b_out", [NL, 1], F32)
    identR = inp("identR", [128, 128])
    onescol = inp("onescol", [128, 1])
    onesrow = inp("onesrow", [1, 128])

    outT = nc.dram_tensor("outT", [NL, NP], F32, kind="ExternalOutput")

    with tile.TileContext(nc) as tc:
        with tc.tile_pool(name="sb", bufs=1) as sb, \
             tc.tile_pool(name="ps", bufs=1, space="PSUM") as ps, \
             tc.tile_pool(name="dr", bufs=1, space="DRAM") as dr:

            ident = sb.tile([128, 128], F32R, name="ident")
            nc.sync.dma_start(ident[:], identR[:])

            def transpose_block(src_ap, pt_shape, dst_ap):
                pt = ps.tile(pt_shape, F32R, name=P.name("ptr"), tag="tr", bufs=2)
                idn = ident[0:pt_shape[1], 0:pt_shape[1]]
                nc.tensor.transpose(pt[:], src_ap, idn)
                nc.vector.tensor_copy(dst_ap, pt[:])

            # ============ A: zz1 ============
            zz1_sl = dr.tile([JH // 8, NP], F32R, name="zz1_sl")
            bt = load_bias(P, sb, b_jb1, JH // 8)
            tp_gemm(P, sb, ps, [(w_jw1, INS)], [(xgT, INS)], JH // 8,
                    act_epilogue(P, sb, zz1_sl, bt, AF.Relu))
            zz1_full = allgather(P, dr, zz1_sl, [JH, NP], "zz1_full")

            # ============ B: zzT slice ============
            zzT = sb.tile([128, 2 * NP], F32R, name="zzT", tag="gxpart")
            bt2 = load_bias(P, sb, b_jb2, S)

            def zz_out(mi, m_off, m_sz, n_off, n_sz):
                return bv(zzT, mi, n_off, n_sz, rows=m_sz)
            tp_gemm(P, sb, ps, [(w_jw2, JH)], [(zz1_full, JH)], S,
                    act_epilogue(P, sb, None, bt2, AF.Relu, out_sb_fn=zz_out))

            # ============ C: deg / dinv ============
            ones_sl = sb.tile([128, 1], F32R, name="ones_sl")
            nc.sync.dma_start(ones_sl[:], onescol[:])
            deg_sb = sb.tile([1, NP], F32, name="deg_sb")
            for (n_off, n_sz) in NC3:
                dps = ps.tile([1, n_sz], F32, name=P.name("dps"), tag="tr", bufs=2)
                nc.tensor.matmul(dps[:], ones_sl[0:128, :], bv(zzT, 0, n_off, n_sz),
                                 start=True, stop=False)
                nc.tensor.matmul(dps[:], ones_sl[0:16, :], bv(zzT, 1, n_off, n_sz),
                                 start=False, stop=True)
                nc.vector.tensor_copy(deg_sb[:, n_off:n_off + n_sz], dps[:])
            deg_bin = dr.tile([1, NP], F32, name="deg_bin")
            nc.gpsimd.dma_start(deg_bin[:], deg_sb[:])
            deg_full = dr.tile([1, NP], F32, name="deg_full", addr_space="Shared")
            nc.gpsimd.collective_compute(
                "AllReduce", mybir.AluOpType.add,
                replica_groups=[list(range(NCORES))],
                ins=[deg_bin.opt()], outs=[deg_full.opt()])
            dinv_f = sb.tile([1, NP], F32, name="dinv_f")
            vm = sb.tile([1, NP], F32, name="vm")
            nc.sync.dma_start(vm[:], vmask[:])
            nc.sync.dma_start(dinv_f[:], deg_full[:])
            nc.vector.tensor_scalar_add(dinv_f[:], dinv_f[:], 1.0)
            nc.vector.reciprocal(dinv_f[:], dinv_f[:])
            nc.scalar.activation(dinv_f[:], dinv_f[:], AF.Sqrt)
            nc.vector.tensor_mul(dinv_f[:], dinv_f[:], vm[:])

            onesr = sb.tile([1, 128], F32R, name="onesr")
            nc.sync.dma_start(onesr[:], onesrow[:])
            dinv_fr = sb.tile([1, NP], F32R, name="dinv_fr")
            nc.vector.tensor_copy(dinv_fr[:], dinv_f[:])
            dinv_b = sb.tile([128, NP], F32R, name="dinv_b", tag="hT")
            for (n_off, n_sz) in NC3:
                bps = ps.tile([128, n_sz], F32, name=P.name("bps"), tag="tr", bufs=2)
                nc.tensor.matmul(bps[:], onesr[:], dinv_fr[:, n_off:n_off + n_sz],
                                 start=True, stop=True)
                nc.vector.tensor_copy(dinv_b[:, n_off:n_off + n_sz], bps[:])

            eyeT_sb = sb.tile([128, 2 * NP], F32R, name="eyeT_sb", tag="h1sb")
            nc.sync.dma_start(bv(eyeT_sb, 0), eyeT[0:128, :])
            nc.sync.dma_start(bv(eyeT_sb, 1), eyeT[128:S, :])
            dinv_p = sb.tile([128, 2], F32, name="dinv_p")
            tmpm = sb.tile([128, NP], F32R, name="tmpm", tag="scratch")
            for bi, ro, rs in BLKS:
                nc.vector.tensor_mul(tmpm[0:rs, :], bv(eyeT_sb, bi), dinv_b[0:rs, :])
                nc.vector.reduce_sum(dinv_p[0:rs, bi:bi + 1], tmpm[0:rs, :],
                                     axis=mybir.AxisListType.X)

            # ============ D: G slice + V init ============
            g_sl = sb.tile([128, 2 * NP], F32R, name="g_sl0")
            v_sl = sb.tile([128, 2 * NP], F32R, name="v_sl0")
            for bi, ro, rs in BLKS:
                g = bv(g_sl, bi)
                nc.vector.tensor_add(g, bv(zzT, bi), bv(eyeT_sb, bi))
                nc.vector.tensor_scalar_mul(g, g, dinv_p[0:rs, bi:bi + 1])
                nc.vector.tensor_mul(g, g, dinv_b[0:rs, :])
                nc.vector.tensor_scalar_mul(g, g, 0.95)
                nc.vector.tensor_add(bv(v_sl, bi), bv(eyeT_sb, bi), g)

            # ====== g1gx: gx part of GCN1 pre-agg (overlaps GDC chain) ======
            W1 = H1 // 8
            gxpart = sb.tile([128, 2 * NP], F32R, name="gxpart", tag="gxpart")
            g1x_sb = sb.tile([128, 32 * W1], F32R, name="g1x_sb", tag="kxmC")
            for kt in range(32):
                nc.sync.dma_start(g1x_sb[:, kt * W1:(kt + 1) * W1],
                                  w_g1gx[kt * 128:(kt + 1) * 128, :])
            for (n_off, n_sz) in NCF:
                pgx = [ps.tile([128, n_sz], F32, name=P.name("pgx"), tag="pA", bufs=2)
                       for _ in range(2)]
                for kt in range(32):
                    rt = sb.tile([128, n_sz], F32R, name=P.name("gxr"), tag="rhs", bufs=4)
                    nc.sync.dma_start(rt[:], xgT[DX + kt * 128: DX + (kt + 1) * 128,
                                                 n_off:n_off + n_sz])
                    for i in range(2):
                        nc.tensor.matmul(
                            pgx[i][:],
                            g1x_sb[:, kt * W1 + i * 128: kt * W1 + i * 128 + 128],
                            rt[:], start=(kt == 0), stop=(kt == 31))
                for i in range(2):
                    nc.vector.tensor_copy(
                        gxpart[0:128, i * NP + n_off: i * NP + n_off + n_sz], pgx[i][:])

            # ============ E: doubling chain ============
            gT = sb.tile([128, 9 * S], F32R, name="gT")
            vT = sb.tile([128, 9 * S], F32R, name="vT")

            def transpose_slice(src_bt, dst_sb):
                for kb in range(9):
                    transpose_block(bv(src_bt, 0, kb * 128, 128), [128, 128],
                                    dst_sb[:, kb * S: kb * S + 128])
                    transpose_block(bv(src_bt, 1, kb * 128, 128), [128, 16],
                                    dst_sb[:, kb * S + 128: (kb + 1) * S])

            for j in range(1, NSQ + 2):
                last = (j == NSQ + 1)
                transpose_slice(g_sl, gT)
                if j > 1:
                    transpose_slice(v_sl, vT)
                gb = dr.tile([S, NP], F32R, name=P.name("g_bin"), tag="g_bin", bufs=2)
                nc.gpsimd.dma_start(gb[0:128, :], bv(g_sl, 0))
                nc.gpsimd.dma_start(gb[128:144, :], bv(g_sl, 1))
                g_full = dr.tile([NP, NP], F32R, name=P.name("g_full"),
                                 tag="g_full", bufs=2, addr_space="Shared")
                nc.gpsimd.collective_compute(
                    "AllGather", mybir.AluOpType.bypass,
                    replica_groups=[list(range(NCORES))],
                    ins=[gb.opt()], outs=[g_full.opt()])

                for (n_off, n_sz) in NC3:
                    pg0 = ps.tile([128, n_sz], F32, name=P.name("pg0"), tag="pB", bufs=1)
                    pg1 = ps.tile([16, n_sz], F32, name=P.name("pg1"), tag="pC", bufs=1)
                    pv0 = ps.tile([128, n_sz], F32, name=P.name("pv0"), tag="pD", bufs=1)
                    pv1 = ps.tile([16, n_sz], F32, name=P.name("pv1"), tag="pE", bufs=1)
                    for kb in range(9):
                        rt = sb.tile([128, n_sz], F32R, name=P.name("grhs"),
                                     tag="grhs", bufs=4)
                        nc.sync.dma_start(rt[:], g_full[kb * 128:(kb + 1) * 128,
                                                        n_off:n_off + n_sz])
                        st, sp = (kb == 0), (kb == 8)
                        if not last:
                            nc.tensor.matmul(pg0[:], gT[:, kb * S: kb * S + 128],
                                             rt[:], start=st, stop=sp)
                            nc.tensor.matmul(pg1[:], gT[:, kb * S + 128:(kb + 1) * S],
                                             rt[:], start=st, stop=sp)
                        if j > 1:
                            nc.tensor.matmul(pv0[:], vT[:, kb * S: kb * S + 128],
                                             rt[:], start=st, stop=sp)
                            nc.tensor.matmul(pv1[:], vT[:, kb * S + 128:(kb + 1) * S],
                                             rt[:], start=st, stop=sp)
                    pgs, pvs = [pg0, pg1], [pv0, pv1]
                    for bi, ro, rs in BLKS:
                        if j > 1:
                            nc.vector.tensor_add(bv(v_sl, bi, n_off, n_sz),
                                                 bv(v_sl, bi, n_off, n_sz), pvs[bi][:])
                        if not last:
                            nc.vector.tensor_copy(bv(g_sl, bi, n_off, n_sz), pgs[bi][:])

            # ============ F: topk + column normalize ============
            vf = sb.tile([128, 2 * NP], F32, name="vf", tag="hT")
            work = sb.tile([128, 2 * NP], F32, name="tkwork", tag="scratch")
            mx = sb.tile([128, 8], F32, name="tkmax")
            for bi, ro, rs in BLKS:
                nc.vector.tensor_copy(bv(vf, bi), bv(v_sl, bi))
            for bi, ro, rs in BLKS:
                cur = bv(vf, bi)
                w = bv(work, bi)
                for it in range(TOPK // 8):
                    nc.vector.max(mx[0:rs, :], cur)
                    nc.vector.match_replace(w, mx[0:rs, :], cur, 0.0)
                    cur = w
            csum = sb.tile([128, 2], F32, name="csum")
            for bi, ro, rs in BLKS:
                nc.vector.tensor_sub(bv(work, bi), bv(vf, bi), bv(work, bi))
                nc.vector.reduce_sum(csum[0:rs, bi:bi + 1], bv(work, bi),
                                     axis=mybir.AxisListType.X)
            nc.vector.tensor_scalar_add(csum[:], csum[:], 1e-30)
            nc.vector.reciprocal(csum[:], csum[:])
            for bi, ro, rs in BLKS:
                nc.vector.tensor_scalar_mul(bv(work, bi), bv(work, bi),
                                            csum[0:rs, bi:bi + 1])
            sn_bin = dr.tile([S, NP], F32R, name="sn_bin")
            nc.gpsimd.dma_start(sn_bin[0:128, :], bv(work, 0))
            nc.gpsimd.dma_start(sn_bin[128:144, :], bv(work, 1))
            snT_full = allgather(P, dr, sn_bin, [NP, NP], "snT_full")

            # ====== Hg: ec1 gx-half -> DRAM partial (fills GDC/topk gaps) ======
            bt_ec1 = load_bias(P, sb, b_ec1, H0 // 8)
            zpart_sl = dr.tile([H0 // 8, NP], F32R, name="zpart_sl")
            tp_gemm(P, sb, ps, [(w_ec1g, DX)], [(xgT[DX:INS, :], DX)],
                    H0 // 8, act_epilogue(P, sb, zpart_sl, None, AF.Copy))

            # ============ G: xn (pD/pE, 2 m-tiles at a time) ============
            xnT_sl = dr.tile([DX // 8, NP], F32R, name="xnT_sl")
            xk_sb = sb.tile([128, 9 * 512], F32R, name="xk_sb", tag="xk")
            for kt in range(9):
                nc.sync.dma_start(xk_sb[:, kt * 512:(kt + 1) * 512],
                                  xcol[kt * 128:(kt + 1) * 128, :])
            for half in range(2):
                for (n_off, n_sz) in NCF:
                    px = [ps.tile([128, n_sz], F32, name=P.name("px"), tag=t, bufs=1)
                          for t in ("pD", "pE")]
                    for kt in range(9):
                        rt = sb.tile([128, n_sz], F32R, name=P.name("snr"),
                                     tag="grhs", bufs=4)
                        nc.sync.dma_start(rt[:], snT_full[kt * 128:(kt + 1) * 128,
                                                          n_off:n_off + n_sz])
                        for i in range(2):
                            mo = half * 256 + i * 128
                            nc.tensor.matmul(px[i][:],
                                             xk_sb[:, kt * 512 + mo: kt * 512 + mo + 128],
                                             rt[:], start=(kt == 0), stop=(kt == 8))
                    for i in range(2):
                        mo = half * 256 + i * 128
                        t = sb.tile([128, n_sz], F32R, name=P.name("epx"), tag="ep",
                                    bufs=3)
                        nc.scalar.activation(t[:], px[i][:], AF.Copy)
                        nc.sync.dma_start(xnT_sl[mo:mo + 128, n_off:n_off + n_sz], t[:])
            xnT_full = allgather(P, dr, xnT_sl, [DX, NP], "xnT_full")

            # ============ Hx: ec1 xn-half + partial + bias/relu ============
            zT_sl = dr.tile([H0 // 8, NP], F32R, name="zT_sl")

            def ep_hx(mi, m_off, m_sz, n_off, n_sz, psum):
                pp = sb.tile([m_sz, n_sz], F32R, name=P.name("pp"), tag="ep", bufs=3)
                nc.sync.dma_start(pp[:], zpart_sl[m_off:m_off + m_sz,
                                                  n_off:n_off + n_sz])
                t = sb.tile([m_sz, n_sz], F32R, name=P.name("epz"), tag="ep", bufs=3)
                nc.vector.tensor_add(t[:], psum[:], pp[:])
                nc.scalar.activation(t[:], t[:], AF.Relu,
                                     bias=bt_ec1[0:m_sz, mi:mi + 1])
                nc.sync.dma_start(zT_sl[m_off:m_off + m_sz, n_off:n_off + n_sz], t[:])
            tp_gemm(P, sb, ps, [(w_ec1x, DX)], [(xnT_full, DX)], H0 // 8, ep_hx)
            zT_full = allgather(P, dr, zT_sl, [H0, NP], "zT_full")

            # ============ zpass: [z0 | zc-z | h1-z] over one zT stream ======
            z0_sb = sb.tile([128, NP], F32R, name="z0_sb")
            zc_acc = sb.tile([128, NP], F32R, name="zc_acc")
            hT_tmp = sb.tile([128, 2 * NP], F32R, name="hT_tmp", tag="hT")
            bt_dr = load_bias(P, sb, b_dr, H2 // 8)

            def ep_zpass(mi, m_off, m_sz, n_off, n_sz, psum):
                if mi == 0:
                    nc.vector.tensor_scalar_add(z0_sb[:, n_off:n_off + n_sz], psum[:],
                                                bt_dr[:, 0:1])
                elif mi == 1:
                    nc.vector.tensor_copy(zc_acc[:, n_off:n_off + n_sz], psum[:])
                else:
                    bi = mi - 2
                    nc.vector.tensor_add(
                        hT_tmp[:, bi * NP + n_off: bi * NP + n_off + n_sz],
                        gxpart[:, bi * NP + n_off: bi * NP + n_off + n_sz], psum[:])
            tp_gemm(P, sb, ps, [(w_zp, H0)], [(zT_full, H0)], 512, ep_zpass)

            h1_sb = sb.tile([128, 9 * W1], F32R, name="h1_sb", tag="h1sb")
            for kb in range(9):
                for i in range(2):
                    transpose_block(
                        hT_tmp[0:128, i * NP + kb * 128: i * NP + (kb + 1) * 128],
                        [128, 128],
                        h1_sb[:, kb * W1 + i * 128: kb * W1 + (i + 1) * 128])

            # ============ z1 agg ============
            z1_sl = dr.tile([H1 // 8, NP], F32R, name="z1_sl")
            bt_g1 = load_bias(P, sb, b_g1, H1 // 8)
            ep_z1 = act_epilogue(P, sb, z1_sl, bt_g1, AF.Relu)
            for (n_off, n_sz) in NCF:
                pz0 = ps.tile([128, n_sz], F32, name=P.name("pz0"), tag="pA", bufs=2)
                pz1 = ps.tile([128, n_sz], F32, name=P.name("pz1"), tag="pA", bufs=2)
                for kb in range(9):
                    rtt = sb.tile([128, n_sz], F32R, name=P.name("ahr"), tag="rhs", bufs=4)
                    nc.sync.dma_start(rtt[:], ahatT[kb * 128:(kb + 1) * 128,
                                                    n_off:n_off + n_sz])
                    st, sp = (kb == 0), (kb == 8)
                    nc.tensor.matmul(pz0[:], h1_sb[:, kb * W1: kb * W1 + 128], rtt[:],
                                     start=st, stop=sp)
                    nc.tensor.matmul(pz1[:], h1_sb[:, kb * W1 + 128:(kb + 1) * W1],
                                     rtt[:], start=st, stop=sp)
                ep_z1(0, 0, 128, n_off, n_sz, pz0)
                ep_z1(1, 128, 128, n_off, n_sz, pz1)
            z1_full = allgather(P, dr, z1_sl, [H1, NP], "z1_full")

            # ============ z1pass: [zc-z1 | h2-z1] ============
            W2 = H2 // 8

            def ep_z1pass(mi, m_off, m_sz, n_off, n_sz, psum):
                if mi == 0:
                    nc.vector.tensor_add(zc_acc[:, n_off:n_off + n_sz],
                                         zc_acc[:, n_off:n_off + n_sz], psum[:])
                else:
                    nc.vector.tensor_copy(hT_tmp[0:128, n_off:n_off + n_sz], psum[:])
            tp_gemm(P, sb, ps, [(w_z1p, H1)], [(z1_full, H1)], 256, ep_z1pass)
            h2_sb = sb.tile([128, 9 * W2], F32R, name="h2_sb")
            for kb in range(9):
                transpose_block(hT_tmp[0:128, kb * 128:(kb + 1) * 128], [128, 128],
                                h2_sb[:, kb * W2:(kb + 1) * W2])

            # ============ z2 agg ============
            z2_sl = dr.tile([H2 // 8, NP], F32R, name="z2_sl")
            bt_g2 = load_bias(P, sb, b_g2, H2 // 8)
            ep_z2 = act_epilogue(P, sb, z2_sl, bt_g2, AF.Relu)
            for (n_off, n_sz) in NCF:
                pz = ps.tile([128, n_sz], F32, name=P.name("pz2"), tag="pA", bufs=2)
                for kb in range(9):
                    rtt = sb.tile([128, n_sz], F32R, name=P.name("ahr2"), tag="rhs",
                                  bufs=4)
                    nc.sync.dma_start(rtt[:], ahatT[kb * 128:(kb + 1) * 128,
                                                    n_off:n_off + n_sz])
                    nc.tensor.matmul(pz[:], h2_sb[:, kb * W2:(kb + 1) * W2], rtt[:],
                                     start=(kb == 0), stop=(kb == 8))
                ep_z2(0, 0, 128, n_off, n_sz, pz)
            z2_full = allgather(P, dr, z2_sl, [H2, NP], "z2_full")

            # ============ zc final + z0 ============
            zcz0_sl = dr.tile([H2 // 8, NP], F32R, name="zcz0_sl")
            bt_ec2 = load_bias(P, sb, b_ec2, H2 // 8)

            def ep_zc(mi, m_off, m_sz, n_off, n_sz, psum):
                t = sb.tile([m_sz, n_sz], F32R, name=P.name("epc"), tag="ep", bufs=3)
                nc.vector.tensor_add(t[:], psum[:], zc_acc[:, n_off:n_off + n_sz])
                nc.scalar.activation(t[:], t[:], AF.Relu, bias=bt_ec2[0:m_sz, 0:1])
                nc.vector.tensor_add(t[:], t[:], z0_sb[0:m_sz, n_off:n_off + n_sz])
                nc.sync.dma_start(zcz0_sl[m_off:m_off + m_sz, n_off:n_off + n_sz], t[:])
            tp_gemm(P, sb, ps, [(w_ec2c, H2)], [(z2_full, H2)], H2 // 8, ep_zc)
            zcz0_full = allgather(P, dr, zcz0_sl, [H2, NP], "zcz0_full")

            # ============ zf ============
            zf_sl = dr.tile([OUTS // 8, NP], F32R, name="zf_sl")
            bt_ec3 = load_bias(P, sb, b_ec3, OUTS // 8)
            tp_gemm(P, sb, ps, [(w_ec3, H2)], [(zcz0_full, H2)], OUTS // 8,
                    act_epilogue(P, sb, zf_sl, bt_ec3, AF.Relu))
            zf_full = allgather(P, dr, zf_sl, [OUTS, NP], "zf_full")

            # ============ out ============
            bt_out = load_bias(P, sb, b_out, NL)

            def ep_out(mi, m_off, m_sz, n_off, n_sz, psum):
                t = sb.tile([m_sz, n_sz], F32, name=P.name("epo"), tag="ep", bufs=3)
                nc.vector.tensor_scalar_add(t[:], psum[:], bt_out[0:m_sz, mi:mi + 1])
                nc.sync.dma_start(outT[m_off:m_off + m_sz, n_off:n_off + n_sz], t[:])
            tp_gemm(P, sb, ps, [(w_out, OUTS)], [(zf_full, OUTS)], NL, ep_out)

    nc.compile()
    legalize_matmul_waits(nc)
    return nc


def shard_inputs(x, gx, edge_index, jw1, jb1, jw2, jb2, ec1_w, ec1_b, dr_w, dr_b,
                 g1_w, g1_b, g2_w, g2_b, ec2_w, ec2_b, ec3_w, ec3_b, out_w, out_b):
    f32 = np.float32
    x = np.asarray(x); gx = np.asarray(gx)
    xp = np.zeros((NP, DX), f32); xp[:N] = x
    gxp = np.zeros((NP, DX), f32); gxp[:N] = gx
    xgT = np.concatenate([xp.T, gxp.T], axis=0).copy()

    row, col = np.asarray(edge_index[0]), np.asarray(edge_index[1])
    deg = np.bincount(col, minlength=N).astype(f32) + 1.0
    dinv = (1.0 / np.sqrt(deg)).astype(f32)
    ahT = np.zeros((NP, NP), f32)
    np.add.at(ahT, (row, col), (dinv[row] * dinv[col]).astype(f32))
    ahT[np.arange(N), np.arange(N)] += dinv * dinv

    jw2p = np.zeros((JH, NP), f32); jw2p[:, :N] = jw2
    jb2p = np.zeros((NP,), f32); jb2p[:N] = jb2
    vmask = np.zeros((1, NP), f32); vmask[0, :N] = 1.0

    def cseg(w, c, width):
        return w[:, c * width:(c + 1) * width]

    ins = []
    for c in range(NCORES):
        cs = slice(c * S, (c + 1) * S)
        eyeT = np.zeros((S, NP), f32)
        rr = np.arange(c * S, min((c + 1) * S, N))
        eyeT[rr - c * S, rr] = 1.0
        w_zp = np.concatenate([cseg(dr_w, c, H2 // 8),
                               cseg(ec2_w[:DX], c, H2 // 8),
                               cseg(g1_w, c, H1 // 8)], axis=1)
        w_z1p = np.concatenate([cseg(ec2_w[DX:DX + H1], c, H2 // 8),
                                cseg(g2_w, c, H2 // 8)], axis=1)
        d = dict(
            xgT=xgT,
            xcol=xp[:, c * (DX // 8):(c + 1) * (DX // 8)],
            ahatT=ahT,
            eyeT=eyeT,
            vmask=vmask,
            w_jw1=cseg(jw1, c, JH // 8),
            b_jb1=np.asarray(jb1)[c * (JH // 8):(c + 1) * (JH // 8)].reshape(-1, 1),
            w_jw2=jw2p[:, cs],
            b_jb2=jb2p[cs].reshape(-1, 1),
            w_ec1x=cseg(ec1_w[:DX], c, H0 // 8),
            w_ec1g=cseg(ec1_w[DX:], c, H0 // 8),
            b_ec1=np.asarray(ec1_b)[c * (H0 // 8):(c + 1) * (H0 // 8)].reshape(-1, 1),
            w_zp=w_zp,
            b_dr=np.asarray(dr_b)[c * (H2 // 8):(c + 1) * (H2 // 8)].reshape(-1, 1),
            w_g1gx=cseg(g1_w, c, H1 // 8),
            b_g1=np.asarray(g1_b)[c * (H1 // 8):(c + 1) * (H1 // 8)].reshape(-1, 1),
            w_z1p=w_z1p,
            b_g2=np.asarray(g2_b)[c * (H2 // 8):(c + 1) * (H2 // 8)].reshape(-1, 1),
            w_ec2c=cseg(ec2_w[DX + H1:], c, H2 // 8),
            b_ec2=np.asarray(ec2_b)[c * (H2 // 8):(c + 1) * (H2 // 8)].reshape(-1, 1),
            w_ec3=cseg(ec3_w, c, OUTS // 8),
            b_ec3=np.asarray(ec3_b)[c * (OUTS // 8):(c + 1) * (OUTS // 8)].reshape(-1, 1),
            w_out=out_w,
            b_out=np.asarray(out_b).reshape(-1, 1),
            identR=np.eye(128, dtype=f32),
            onescol=np.ones((128, 1), f32),
            onesrow=np.ones((1, 128), f32),
        )
        ins.append({k: np.ascontiguousarray(v, dtype=f32) for k, v in d.items()})
    return ins


_PROG = [None]


def kernel(**inputs) -> np.ndarray:
    in_maps = shard_inputs(**inputs)
    if _PROG[0] is None:
        _PROG[0] = build_program()
    nc = _PROG[0]
    res = bass_utils.run_bass_kernel_spmd(nc, in_maps, core_ids=list(range(NCORES)))
    outT = res.results[0]["outT"]
    return np.ascontiguousarray(outT[:, :N].T)


# revision 21
# speedup vs baseline: 3.0102x; 3.0102x over previous
"""Trainium2 Bass kernel for nn_AU_Net_3573412790684 (GNN message passing).

Strategy (8 NeuronCores, SPMD + collectives):
  - Node dim padded 1026 -> NP=1152 (9*128); nodes sharded 144/core.
  - Activations feature-major (X^T layout); weight-column tensor-parallel
    GEMMs with AllGather of activation slices between layers.
  - GDC exact PPR via Neumann doubling on G = M^T (row-sharded);
    top-128 per S-column via DVE max8/match_replace; column normalize.
  - GCN layers as dense matmuls vs host-built AhatT; their node-major lhsT
    operands produced by PE tile transposes of feature-major results.
  - All matmul operands float32r (fp32 storage, fast PE mode at N>=256).
  - Fused passes: one zT stream serves dr_w + ec2[:4096] + g1_w; one z1T
    stream serves ec2[4096:6144] + g2_w.

Per-core 144-row state in [128, 2*NP] block tiles (block1 = rows 128..143 in
partitions 0..15).  PSUM tags: pA(bufs2) pB pC pD pE(bufs1) tr(bufs2) = 8 banks.
"""
import sys
import numpy as np

sys.path.insert(0, "/opt/trn_rl_repo")
import concourse.bass as bass
from concourse import bacc
import concourse.mybir as mybir
import concourse.tile as tile
from concourse import bass_utils

import bass_rust

_SKIP_WAIT_SPLIT = ("InstDrain", "InstCollectiveCompute", "InstEventSemaphore",
                    "InstCall", "InstHalt", "InstAllEngineBarrier",
                    "InstBranchHint")
_ev_uid = [0]


def legalize_matmul_waits(nc, max_waits: int = 1):
    """walrus rejects instructions carrying more than one sync-wait command;
    split excess waits into standalone same-engine InstEventSemaphores."""
    moved = 0
    for f in nc.m.functions:
        for bb in f.blocks:
            out = []
            for ins in bb.instructions:
                tn = type(ins).__name__
                si = ins.sync_info
                if (si is not None and len(si.on_wait) > max_waits
                        and tn not in _SKIP_WAIT_SPLIT):
                    for w in list(si.on_wait):
                        _ev_uid[0] += 1
                        ev = mybir.InstEventSemaphore(
                            name=f"waitev-{_ev_uid[0]}", ins=[], outs=[])
                        ev.engine = ins.engine
                        ev.sync_info = bass_rust.SyncInfo(on_wait=[w], on_update=[])
                        ev.bass_nofuse = True
                        out.append(ev)
                    ins.sync_info = bass_rust.SyncInfo(
                        on_wait=[], on_update=list(si.on_update))
                    moved += 1
                out.append(ins)
            bb.instructions[:] = out
    return moved

F32 = mybir.dt.float32
F32R = mybir.dt.float32r
AF = mybir.ActivationFunctionType

N = 1026
NP = 1152
S = NP // 8
DX = 4096
INS = 8192
JH = 2048
H0 = 4096
H1 = 2048
H2 = 1024
OUTS = 512
NL = 10
TOPK = 128
NSQ = 8
NCORES = 8
NC3 = [(0, 384), (384, 384), (768, 384)]     # full width (GDC chain)
NCF = [(0, 384), (384, 384), (768, 258)]     # feature gemms: skip pad cols
BLKS = [(0, 0, 128), (1, 128, 16)]

PS_TAGS = ["pA", "pA", "pB", "pC"]           # tp_gemm m-tile psum tags
PS_BUFS = [2, 2, 1, 1]


def _ceil(a, b):
    return -(-a // b)


def _mtiles(M):
    out, o = [], 0
    while o < M:
        t = min(128, M - o)
        out.append((o, t))
        o += t
    return out


class Prog:
    def __init__(self):
        self.nc = bacc.Bacc("TRN2", target_bir_lowering=False, debug=False,
                            num_devices=NCORES)
        self.uid = 0

    def name(self, p):
        self.uid += 1
        return f"{p}_{self.uid}"


def bv(t, bi, n_off=0, n_sz=NP, rows=None):
    r = (128 if bi == 0 else 16) if rows is None else rows
    return t[0:r, bi * NP + n_off: bi * NP + n_off + n_sz]


def tp_gemm(P, sb, ps, kxm_srcs, kxn_srcs, M, epilogue, n_chunks=NCF,
            cache_kxm=True, carry_in=None, carry_out=False):
    """out[M, n] += kxm^T @ kxn.  carry_in/carry_out: split-K across calls."""
    nc = P.nc
    ktiles = []
    for si, (ap, rows) in enumerate(kxm_srcs):
        for r in range(0, rows, 128):
            ktiles.append((si, r))
    nkt = len(ktiles)
    rh = []
    for si, (ap, rows) in enumerate(kxn_srcs):
        for r in range(0, rows, 128):
            rh.append((si, r))
    assert len(rh) == nkt
    mts = _mtiles(M)

    kxm_sb = None
    if cache_kxm:
        kxm_sb = sb.tile([128, nkt * M], F32R, name=P.name("kxmC"), tag="kxmC")
        kt = 0
        while kt < nkt:
            si, r = ktiles[kt]
            nb = 1
            while (nb < 4 and kt + nb < nkt and ktiles[kt + nb][0] == si
                   and ktiles[kt + nb][1] == r + nb * 128):
                nb += 1
            nc.sync.dma_start(
                kxm_sb[:, kt * M:(kt + nb) * M],
                kxm_srcs[si][0][r:r + nb * 128, :].rearrange("(a p) m -> p a m", p=128))
            kt += nb

    # batch consecutive k-tiles of the same rhs source into one DMA
    KB = 4
    batches = []
    kt = 0
    while kt < nkt:
        si, r = rh[kt]
        nb = 1
        while (nb < KB and kt + nb < nkt and rh[kt + nb][0] == si
               and rh[kt + nb][1] == r + nb * 128):
            nb += 1
        batches.append((si, r, nb, kt))
        kt += nb

    psums_all = carry_in if carry_in is not None else {}
    for ci, (n_off, n_sz) in enumerate(n_chunks):
        if carry_in is not None:
            psums = psums_all[ci]
        else:
            psums = [ps.tile([m_sz, n_sz], F32, name=P.name("psg"),
                             tag=PS_TAGS[mi], bufs=PS_BUFS[mi])
                     for mi, (m_off, m_sz) in enumerate(mts)]
            psums_all[ci] = psums
        for (si, r, nb, kt0) in batches:
            rt = sb.tile([128, KB * n_sz], F32R, name=P.name("rhs"), tag="rhs", bufs=2)
            src = kxn_srcs[si][0][r:r + nb * 128, n_off:n_off + n_sz]
            nc.sync.dma_start(rt[0:128, 0:nb * n_sz],
                              src.rearrange("(a p) n -> p a n", p=128))
            for kk in range(nb):
                kt = kt0 + kk
                for mi, (m_off, m_sz) in enumerate(mts):
                    lh = kxm_sb[:, kt * M + m_off: kt * M + m_off + m_sz]
                    nc.tensor.matmul(psums[mi][:], lh,
                                     rt[:, kk * n_sz:(kk + 1) * n_sz],
                                     start=(kt == 0 and carry_in is None),
                                     stop=(kt == nkt - 1 and not carry_out))
        if not carry_out:
            for mi, (m_off, m_sz) in enumerate(mts):
                epilogue(mi, m_off, m_sz, n_off, n_sz, psums[mi])
    return psums_all


def act_epilogue(P, sb, out_dram, bias_tile, func, out_sb_fn=None):
    nc = P.nc

    def ep(mi, m_off, m_sz, n_off, n_sz, psum):
        t = sb.tile([m_sz, n_sz], F32R, name=P.name("ep"), tag="ep", bufs=3)
        if bias_tile is not None and func == AF.Copy:
            nc.vector.tensor_scalar_add(t[:], psum[:], bias_tile[0:m_sz, mi:mi + 1])
        elif bias_tile is not None:
            nc.scalar.activation(t[:], psum[:], func,
                                 bias=bias_tile[0:m_sz, mi:mi + 1])
        else:
            nc.scalar.activation(t[:], psum[:], func)
        if out_dram is not None:
            nc.sync.dma_start(out_dram[m_off:m_off + m_sz, n_off:n_off + n_sz], t[:])
        if out_sb_fn is not None:
            nc.vector.tensor_copy(out_sb_fn(mi, m_off, m_sz, n_off, n_sz), t[:])
    return ep


def load_bias(P, sb, bias_dram, M):
    nc = P.nc
    t = sb.tile([128, _ceil(M, 128)], F32, name=P.name("bias"),
                tag=P.name("bias"), bufs=1)
    for mi, (m_off, m_sz) in enumerate(_mtiles(M)):
        nc.sync.dma_start(t[:m_sz, mi:mi + 1], bias_dram[m_off:m_off + m_sz, :])
    return t


def allgather(P, dr, slice_dram, full_shape, name):
    nc = P.nc
    full = dr.tile(full_shape, F32R, name=name, addr_space="Shared")
    nc.gpsimd.collective_compute(
        "AllGather", mybir.AluOpType.bypass,
        replica_groups=[list(range(NCORES))],
        ins=[slice_dram.opt()], outs=[full.opt()])
    return full


def build_program():
    P = Prog()
    nc = P.nc

    def inp(name, shape, dt=F32R):
        return nc.dram_tensor(name, shape, dt, kind="ExternalInput")

    xgT = inp("xgT", [INS, NP])
    xcol = inp("xcol", [NP, DX // 8])
    ahatT = inp("ahatT", [NP, NP])
    eyeT = inp("eyeT", [S, NP])
    vmask = inp("vmask", [1, NP], F32)
    w_jw1 = inp("w_jw1", [INS, JH // 8]); b_jb1 = inp("b_jb1", [JH // 8, 1], F32)
    w_jw2 = inp("w_jw2", [JH, S]); b_jb2 = inp("b_jb2", [S, 1], F32)
    w_ec1x = inp("w_ec1x", [DX, H0 // 8])
    w_ec1g = inp("w_ec1g", [DX, H0 // 8]); b_ec1 = inp("b_ec1", [H0 // 8, 1], F32)
    w_zp = inp("w_zp", [H0, 512])            # [dr_w | ec2_w[:DX] | g1_w]
    b_dr = inp("b_dr", [H2 // 8, 1], F32)
    w_g1gx = inp("w_g1gx", [DX, H1 // 8])
    b_g1 = inp("b_g1", [H1 // 8, 1], F32)
    w_z1p = inp("w_z1p", [H1, 256])          # [ec2_w[DX:DX+H1] | g2_w]
    b_g2 = inp("b_g2", [H2 // 8, 1], F32)
    w_ec2c = inp("w_ec2c", [H2, H2 // 8]); b_ec2 = inp("b_ec2", [H2 // 8, 1], F32)
    w_ec3 = inp("w_ec3", [H2, OUTS // 8]); b_ec3 = inp("b_ec3", [OUTS // 8, 1], F32)
    w_out = inp("w_out", [OUTS, NL]); b_out = inp("b_out", [NL, 1], F32)
    identR = inp("identR", [128, 128])
    onescol = inp("onescol", [128, 1])
    onesrow = inp("onesrow", [1, 128])

    outT = nc.dram_tensor("outT", [NL, NP], F32, kind="ExternalOutput")

    with tile.TileContext(nc) as tc:
        with tc.tile_pool(name="sb", bufs=1) as sb, \
             tc.tile_pool(name="ps", bufs=1, space="PSUM") as ps, \
             tc.tile_pool(name="dr", bufs=1, space="DRAM") as dr:

            ident = sb.tile([128, 128], F32R, name="ident")
            nc.sync.dma_start(ident[:], identR[:])

            def transpose_block(src_ap, pt_shape, dst_ap):
                pt = ps.tile(pt_shape, F32R, name=P.name("ptr"), tag="tr", bufs=2)
                idn = ident[0:pt_shape[1], 0:pt_shape[1]]
                nc.tensor.transpose(pt[:], src_ap, idn)
                nc.vector.tensor_copy(dst_ap, pt[:])

            # ============ A: zz1 ============
            zz1_sl = dr.tile([JH // 8, NP], F32R, name="zz1_sl")
            bt = load_bias(P, sb, b_jb1, JH // 8)
            tp_gemm(P, sb, ps, [(w_jw1, INS)], [(xgT, INS)], JH // 8,
                    act_epilogue(P, sb, zz1_sl, bt, AF.Relu))
            zz1_full = allgather(P, dr, zz1_sl, [JH, NP], "zz1_full")

            # ============ B: zzT slice ============
            zzT = sb.tile([128, 2 * NP], F32R, name="zzT", tag="gxpart")
            bt2 = load_bias(P, sb, b_jb2, S)

            def zz_out(mi, m_off, m_sz, n_off, n_sz):
                return bv(zzT, mi, n_off, n_sz, rows=m_sz)
            tp_gemm(P, sb, ps, [(w_jw2, JH)], [(zz1_full, JH)], S,
                    act_epilogue(P, sb, None, bt2, AF.Relu, out_sb_fn=zz_out))

            # ============ C: deg / dinv ============
            ones_sl = sb.tile([128, 1], F32R, name="ones_sl")
            nc.sync.dma_start(ones_sl[:], onescol[:])
            deg_sb = sb.tile([1, NP], F32, name="deg_sb")
            for (n_off, n_sz) in NC3:
                dps = ps.tile([1, n_sz], F32, name=P.name("dps"), tag="tr", bufs=2)
                nc.tensor.matmul(dps[:], ones_sl[0:128, :], bv(zzT, 0, n_off, n_sz),
                                 start=True, stop=False)
                nc.tensor.matmul(dps[:], ones_sl[0:16, :], bv(zzT, 1, n_off, n_sz),
                                 start=False, stop=True)
                nc.vector.tensor_copy(deg_sb[:, n_off:n_off + n_sz], dps[:])
            deg_bin = dr.tile([1, NP], F32, name="deg_bin")
            nc.gpsimd.dma_start(deg_bin[:], deg_sb[:])
            deg_full = dr.tile([1, NP], F32, name="deg_full", addr_space="Shared")
            nc.gpsimd.collective_compute(
                "AllReduce", mybir.AluOpType.add,
                replica_groups=[list(range(NCORES))],
                ins=[deg_bin.opt()], outs=[deg_full.opt()])
            dinv_f = sb.tile([1, NP], F32, name="dinv_f")
            vm = sb.tile([1, NP], F32, name="vm")
            nc.sync.dma_start(vm[:], vmask[:])
            nc.sync.dma_start(dinv_f[:], deg_full[:])
            nc.vector.tensor_scalar_add(dinv_f[:], dinv_f[:], 1.0)
            nc.vector.reciprocal(dinv_f[:], dinv_f[:])
            nc.scalar.activation(dinv_f[:], dinv_f[:], AF.Sqrt)
            nc.vector.tensor_mul(dinv_f[:], dinv_f[:], vm[:])

            onesr = sb.tile([1, 128], F32R, name="onesr")
            nc.sync.dma_start(onesr[:], onesrow[:])
            dinv_fr = sb.tile([1, NP], F32R, name="dinv_fr")
            nc.vector.tensor_copy(dinv_fr[:], dinv_f[:])
            dinv_b = sb.tile([128, NP], F32R, name="dinv_b", tag="hT")
            for (n_off, n_sz) in NC3:
                bps = ps.tile([128, n_sz], F32, name=P.name("bps"), tag="tr", bufs=2)
                nc.tensor.matmul(bps[:], onesr[:], dinv_fr[:, n_off:n_off + n_sz],
                                 start=True, stop=True)
                nc.vector.tensor_copy(dinv_b[:, n_off:n_off + n_sz], bps[:])

            eyeT_sb = sb.tile([128, 2 * NP], F32R, name="eyeT_sb", tag="h1sb")
            nc.sync.dma_start(bv(eyeT_sb, 0), eyeT[0:128, :])
            nc.sync.dma_start(bv(eyeT_sb, 1), eyeT[128:S, :])
            dinv_p = sb.tile([128, 2], F32, name="dinv_p")
            tmpm = sb.tile([128, NP], F32R, name="tmpm", tag="scratch")
            for bi, ro, rs in BLKS:
                nc.vector.tensor_mul(tmpm[0:rs, :], bv(eyeT_sb, bi), dinv_b[0:rs, :])
                nc.vector.reduce_sum(dinv_p[0:rs, bi:bi + 1], tmpm[0:rs, :],
                                     axis=mybir.AxisListType.X)

            # ============ D: G slice + V init ============
            g_sl = sb.tile([128, 2 * NP], F32R, name="g_sl0")
            v_sl = sb.tile([128, 2 * NP], F32R, name="v_sl0")
            for bi, ro, rs in BLKS:
                g = bv(g_sl, bi)
                nc.vector.tensor_add(g, bv(zzT, bi), bv(eyeT_sb, bi))
                nc.vector.tensor_scalar_mul(g, g, dinv_p[0:rs, bi:bi + 1])
                nc.vector.tensor_mul(g, g, dinv_b[0:rs, :])
                nc.vector.tensor_scalar_mul(g, g, 0.95)
                nc.vector.tensor_add(bv(v_sl, bi), bv(eyeT_sb, bi), g)

            # ====== g1gx: gx part of GCN1 pre-agg (overlaps GDC chain) ======
            W1 = H1 // 8
            gxpart = sb.tile([128, 2 * NP], F32R, name="gxpart", tag="gxpart")
            g1x_sb = sb.tile([128, 32 * W1], F32R, name="g1x_sb", tag="kxmC")
            for kt in range(32):
                nc.sync.dma_start(g1x_sb[:, kt * W1:(kt + 1) * W1],
                                  w_g1gx[kt * 128:(kt + 1) * 128, :])
            for (n_off, n_sz) in NCF:
                pgx = [ps.tile([128, n_sz], F32, name=P.name("pgx"), tag="pA", bufs=2)
                       for _ in range(2)]
                for bb in range(8):
                    rt4 = sb.tile([128, 4 * n_sz], F32R, name=P.name("gxr"),
                                  tag="rhs", bufs=2)
                    nc.sync.dma_start(
                        rt4[:],
                        xgT[DX + bb * 512: DX + (bb + 1) * 512, n_off:n_off + n_sz]
                        .rearrange("(a p) n -> p a n", p=128))
                    for kk in range(4):
                        kt = bb * 4 + kk
                        rt = rt4[:, kk * n_sz:(kk + 1) * n_sz]
                        for i in range(2):
                            nc.tensor.matmul(
                                pgx[i][:],
                                g1x_sb[:, kt * W1 + i * 128: kt * W1 + i * 128 + 128],
                                rt, start=(kt == 0), stop=(kt == 31))
                for i in range(2):
                    nc.vector.tensor_copy(
                        gxpart[0:128, i * NP + n_off: i * NP + n_off + n_sz], pgx[i][:])

            # ============ E: doubling chain ============
            gT = sb.tile([128, 9 * S], F32R, name="gT")
            vT = sb.tile([128, 9 * S], F32R, name="vT")

            def transpose_slice(src_bt, dst_sb):
                for kb in range(9):
                    transpose_block(bv(src_bt, 0, kb * 128, 128), [128, 128],
                                    dst_sb[:, kb * S: kb * S + 128])
                    transpose_block(bv(src_bt, 1, kb * 128, 128), [128, 16],
                                    dst_sb[:, kb * S + 128: (kb + 1) * S])

            for j in range(1, NSQ + 2):
                last = (j == NSQ + 1)
                transpose_slice(g_sl, gT)
                if j > 1:
                    transpose_slice(v_sl, vT)
                gb = dr.tile([S, NP], F32R, name=P.name("g_bin"), tag="g_bin", bufs=2)
                nc.gpsimd.dma_start(gb[0:128, :], bv(g_sl, 0))
                nc.gpsimd.dma_start(gb[128:144, :], bv(g_sl, 1))
                g_full = dr.tile([NP, NP], F32R, name=P.name("g_full"),
                                 tag="g_full", bufs=2, addr_space="Shared")
                nc.gpsimd.collective_compute(
                    "AllGather", mybir.AluOpType.bypass,
                    replica_groups=[list(range(NCORES))],
                    ins=[gb.opt()], outs=[g_full.opt()])

                for (n_off, n_sz) in NC3:
                    pg0 = ps.tile([128, n_sz], F32, name=P.name("pg0"), tag="pB", bufs=1)
                    pg1 = ps.tile([16, n_sz], F32, name=P.name("pg1"), tag="pC", bufs=1)
                    pv0 = ps.tile([128, n_sz], F32, name=P.name("pv0"), tag="pD", bufs=1)
                    pv1 = ps.tile([16, n_sz], F32, name=P.name("pv1"), tag="pE", bufs=1)
                    rts = []
                    for bb in range(3):
                        rt3 = sb.tile([128, 3 * n_sz], F32R, name=P.name("grhs"),
                                      tag="grhs", bufs=2)
                        nc.sync.dma_start(
                            rt3[:],
                            g_full[bb * 384: (bb + 1) * 384, n_off:n_off + n_sz]
                            .rearrange("(a p) n -> p a n", p=128))
                        rts.append(rt3)
                    for kb in range(9):
                        rt = rts[kb // 3][:, (kb % 3) * n_sz:(kb % 3 + 1) * n_sz]
                        st, sp = (kb == 0), (kb == 8)
                        if not last:
                            nc.tensor.matmul(pg0[:], gT[:, kb * S: kb * S + 128],
                                             rt, start=st, stop=sp)
                            nc.tensor.matmul(pg1[:], gT[:, kb * S + 128:(kb + 1) * S],
                                             rt, start=st, stop=sp)
                        if j > 1:
                            nc.tensor.matmul(pv0[:], vT[:, kb * S: kb * S + 128],
                                             rt, start=st, stop=sp)
                            nc.tensor.matmul(pv1[:], vT[:, kb * S + 128:(kb + 1) * S],
                                             rt, start=st, stop=sp)
                    pgs, pvs = [pg0, pg1], [pv0, pv1]
                    for bi, ro, rs in BLKS:
                        if j > 1:
                            nc.vector.tensor_add(bv(v_sl, bi, n_off, n_sz),
                                                 bv(v_sl, bi, n_off, n_sz), pvs[bi][:])
                        if not last:
                            nc.vector.tensor_copy(bv(g_sl, bi, n_off, n_sz), pgs[bi][:])

            # ============ F: topk + column normalize ============
            vf = sb.tile([128, 2 * NP], F32, name="vf", tag="hT")
            work = sb.tile([128, 2 * NP], F32, name="tkwork", tag="scratch")
            mx = sb.tile([128, 8], F32, name="tkmax")
            for bi, ro, rs in BLKS:
                nc.vector.tensor_copy(bv(vf, bi), bv(v_sl, bi))
            for bi, ro, rs in BLKS:
                cur = bv(vf, bi)
                w = bv(work, bi)
                for it in range(TOPK // 8):
                    nc.vector.max(mx[0:rs, :], cur)
                    nc.vector.match_replace(w, mx[0:rs, :], cur, 0.0)
                    cur = w
            csum = sb.tile([128, 2], F32, name="csum")
            for bi, ro, rs in BLKS:
                nc.vector.tensor_sub(bv(work, bi), bv(vf, bi), bv(work, bi))
                nc.vector.reduce_sum(csum[0:rs, bi:bi + 1], bv(work, bi),
                                     axis=mybir.AxisListType.X)
            nc.vector.tensor_scalar_add(csum[:], csum[:], 1e-30)
            nc.vector.reciprocal(csum[:], csum[:])
            for bi, ro, rs in BLKS:
                nc.vector.tensor_scalar_mul(bv(work, bi), bv(work, bi),
                                            csum[0:rs, bi:bi + 1])
            sn_bin = dr.tile([S, NP], F32R, name="sn_bin")
            nc.gpsimd.dma_start(sn_bin[0:128, :], bv(work, 0))
            nc.gpsimd.dma_start(sn_bin[128:144, :], bv(work, 1))
            snT_full = allgather(P, dr, sn_bin, [NP, NP], "snT_full")

            # ====== Hg: ec1 gx-half -> DRAM partial (fills GDC/topk gaps) ======
            bt_ec1 = load_bias(P, sb, b_ec1, H0 // 8)
            zpart_sl = dr.tile([H0 // 8, NP], F32R, name="zpart_sl")
            tp_gemm(P, sb, ps, [(w_ec1g, DX)], [(xgT[DX:INS, :], DX)],
                    H0 // 8, act_epilogue(P, sb, zpart_sl, None, AF.Copy))

            # ============ G: xn (pD/pE, 2 m-tiles at a time) ============
            xnT_sl = dr.tile([DX // 8, NP], F32R, name="xnT_sl")
            xk_sb = sb.tile([128, 9 * 512], F32R, name="xk_sb", tag="xk")
            for kt in range(9):
                nc.sync.dma_start(xk_sb[:, kt * 512:(kt + 1) * 512],
                                  xcol[kt * 128:(kt + 1) * 128, :])
            for half in range(2):
                for (n_off, n_sz) in NCF:
                    px = [ps.tile([128, n_sz], F32, name=P.name("px"), tag=t, bufs=1)
                          for t in ("pD", "pE")]
                    rts = []
                    for bb in range(3):
                        rt3 = sb.tile([128, 3 * n_sz], F32R, name=P.name("snr"),
                                      tag="grhs", bufs=2)
                        nc.sync.dma_start(
                            rt3[:],
                            snT_full[bb * 384: (bb + 1) * 384, n_off:n_off + n_sz]
                            .rearrange("(a p) n -> p a n", p=128))
                        rts.append(rt3)
                    for kt in range(9):
                        rt = rts[kt // 3][:, (kt % 3) * n_sz:(kt % 3 + 1) * n_sz]
                        for i in range(2):
                            mo = half * 256 + i * 128
                            nc.tensor.matmul(px[i][:],
                                             xk_sb[:, kt * 512 + mo: kt * 512 + mo + 128],
                                             rt, start=(kt == 0), stop=(kt == 8))
                    for i in range(2):
                        mo = half * 256 + i * 128
                        t = sb.tile([128, n_sz], F32R, name=P.name("epx"), tag="ep",
                                    bufs=3)
                        nc.scalar.activation(t[:], px[i][:], AF.Copy)
                        nc.sync.dma_start(xnT_sl[mo:mo + 128, n_off:n_off + n_sz], t[:])
            xnT_full = allgather(P, dr, xnT_sl, [DX, NP], "xnT_full")

            # ============ Hx: ec1 xn-half + partial + bias/relu ============
            zT_sl = dr.tile([H0 // 8, NP], F32R, name="zT_sl")

            def ep_hx(mi, m_off, m_sz, n_off, n_sz, psum):
                pp = sb.tile([m_sz, n_sz], F32R, name=P.name("pp"), tag="ep", bufs=3)
                nc.sync.dma_start(pp[:], zpart_sl[m_off:m_off + m_sz,
                                                  n_off:n_off + n_sz])
                t = sb.tile([m_sz, n_sz], F32R, name=P.name("epz"), tag="ep", bufs=3)
                nc.vector.tensor_add(t[:], psum[:], pp[:])
                nc.scalar.activation(t[:], t[:], AF.Relu,
                                     bias=bt_ec1[0:m_sz, mi:mi + 1])
                nc.sync.dma_start(zT_sl[m_off:m_off + m_sz, n_off:n_off + n_sz], t[:])
            tp_gemm(P, sb, ps, [(w_ec1x, DX)], [(xnT_full, DX)], H0 // 8, ep_hx)
            zT_full = allgather(P, dr, zT_sl, [H0, NP], "zT_full")

            # ============ zpass: [z0 | zc-z | h1-z] over one zT stream ======
            z0_sb = sb.tile([128, NP], F32R, name="z0_sb")
            zc_acc = sb.tile([128, NP], F32R, name="zc_acc")
            hT_tmp = sb.tile([128, 2 * NP], F32R, name="hT_tmp", tag="hT")
            bt_dr = load_bias(P, sb, b_dr, H2 // 8)

            def ep_zpass(mi, m_off, m_sz, n_off, n_sz, psum):
                if mi == 0:
                    nc.vector.tensor_scalar_add(z0_sb[:, n_off:n_off + n_sz], psum[:],
                                                bt_dr[:, 0:1])
                elif mi == 1:
                    nc.vector.tensor_copy(zc_acc[:, n_off:n_off + n_sz], psum[:])
                else:
                    bi = mi - 2
                    nc.vector.tensor_add(
                        hT_tmp[:, bi * NP + n_off: bi * NP + n_off + n_sz],
                        gxpart[:, bi * NP + n_off: bi * NP + n_off + n_sz], psum[:])
            tp_gemm(P, sb, ps, [(w_zp, H0)], [(zT_full, H0)], 512, ep_zpass)

            h1_sb = sb.tile([128, 9 * W1], F32R, name="h1_sb", tag="h1sb")
            zpad = sb.tile([128, NP - N], F32, name="zpad")
            nc.vector.memset(zpad[:], 0.0)
            nc.vector.tensor_copy(hT_tmp[:, N:NP], zpad[:])
            nc.vector.tensor_copy(hT_tmp[:, NP + N:2 * NP], zpad[:])
            for kb in range(9):
                for i in range(2):
                    transpose_block(
                        hT_tmp[0:128, i * NP + kb * 128: i * NP + (kb + 1) * 128],
                        [128, 128],
                        h1_sb[:, kb * W1 + i * 128: kb * W1 + (i + 1) * 128])

            # ============ z1 agg ============
            z1_sl = dr.tile([H1 // 8, NP], F32R, name="z1_sl")
            bt_g1 = load_bias(P, sb, b_g1, H1 // 8)
            ep_z1 = act_epilogue(P, sb, z1_sl, bt_g1, AF.Relu)
            for (n_off, n_sz) in NCF:
                pz0 = ps.tile([128, n_sz], F32, name=P.name("pz0"), tag="pA", bufs=2)
                pz1 = ps.tile([128, n_sz], F32, name=P.name("pz1"), tag="pA", bufs=2)
                rts = []
                for bb in range(3):
                    rt3 = sb.tile([128, 3 * n_sz], F32R, name=P.name("ahr"),
                                  tag="grhs", bufs=2)
                    nc.sync.dma_start(
                        rt3[:],
                        ahatT[bb * 384: (bb + 1) * 384, n_off:n_off + n_sz]
                        .rearrange("(a p) n -> p a n", p=128))
                    rts.append(rt3)
                for kb in range(9):
                    rtt = rts[kb // 3][:, (kb % 3) * n_sz:(kb % 3 + 1) * n_sz]
                    st, sp = (kb == 0), (kb == 8)
                    nc.tensor.matmul(pz0[:], h1_sb[:, kb * W1: kb * W1 + 128], rtt,
                                     start=st, stop=sp)
                    nc.tensor.matmul(pz1[:], h1_sb[:, kb * W1 + 128:(kb + 1) * W1],
                                     rtt, start=st, stop=sp)
                ep_z1(0, 0, 128, n_off, n_sz, pz0)
                ep_z1(1, 128, 128, n_off, n_sz, pz1)
            z1_full = allgather(P, dr, z1_sl, [H1, NP], "z1_full")

            # ============ z1pass: [zc-z1 | h2-z1] ============
            W2 = H2 // 8

            def ep_z1pass(mi, m_off, m_sz, n_off, n_sz, psum):
                if mi == 0:
                    nc.vector.tensor_add(zc_acc[:, n_off:n_off + n_sz],
                                         zc_acc[:, n_off:n_off + n_sz], psum[:])
                else:
                    nc.vector.tensor_copy(hT_tmp[0:128, n_off:n_off + n_sz], psum[:])
            tp_gemm(P, sb, ps, [(w_z1p, H1)], [(z1_full, H1)], 256, ep_z1pass)
            nc.vector.tensor_copy(hT_tmp[:, N:NP], zpad[:])
            h2_sb = sb.tile([128, 9 * W2], F32R, name="h2_sb")
            for kb in range(9):
                transpose_block(hT_tmp[0:128, kb * 128:(kb + 1) * 128], [128, 128],
                                h2_sb[:, kb * W2:(kb + 1) * W2])

            # ============ z2 agg ============
            z2_sl = dr.tile([H2 // 8, NP], F32R, name="z2_sl")
            bt_g2 = load_bias(P, sb, b_g2, H2 // 8)
            ep_z2 = act_epilogue(P, sb, z2_sl, bt_g2, AF.Relu)
            for (n_off, n_sz) in NCF:
                pz = ps.tile([128, n_sz], F32, name=P.name("pz2"), tag="pA", bufs=2)
                rts = []
                for bb in range(3):
                    rt3 = sb.tile([128, 3 * n_sz], F32R, name=P.name("ahr2"),
                                  tag="grhs", bufs=2)
                    nc.sync.dma_start(
                        rt3[:],
                        ahatT[bb * 384: (bb + 1) * 384, n_off:n_off + n_sz]
                        .rearrange("(a p) n -> p a n", p=128))
                    rts.append(rt3)
                for kb in range(9):
                    rtt = rts[kb // 3][:, (kb % 3) * n_sz:(kb % 3 + 1) * n_sz]
                    nc.tensor.matmul(pz[:], h2_sb[:, kb * W2:(kb + 1) * W2], rtt,
                                     start=(kb == 0), stop=(kb == 8))
                ep_z2(0, 0, 128, n_off, n_sz, pz)
            z2_full = allgather(P, dr, z2_sl, [H2, NP], "z2_full")

            # ============ zc final + z0 ============
            zcz0_sl = dr.tile([H2 // 8, NP], F32R, name="zcz0_sl")
            bt_ec2 = load_bias(P, sb, b_ec2, H2 // 8)

            def ep_zc(mi, m_off, m_sz, n_off, n_sz, psum):
                t = sb.tile([m_sz, n_sz], F32R, name=P.name("epc"), tag="ep", bufs=3)
                nc.vector.tensor_add(t[:], psum[:], zc_acc[:, n_off:n_off + n_sz])
                nc.scalar.activation(t[:], t[:], AF.Relu, bias=bt_ec2[0:m_sz, 0:1])
                nc.vector.tensor_add(t[:], t[:], z0_sb[0:m_sz, n_off:n_off + n_sz])
                nc.sync.dma_start(zcz0_sl[m_off:m_off + m_sz, n_off:n_off + n_sz], t[:])
            tp_gemm(P, sb, ps, [(w_ec2c, H2)], [(z2_full, H2)], H2 // 8, ep_zc)
            zcz0_full = allgather(P, dr, zcz0_sl, [H2, NP], "zcz0_full")

            # ============ zf ============
            zf_sl = dr.tile([OUTS // 8, NP], F32R, name="zf_sl")
            bt_ec3 = load_bias(P, sb, b_ec3, OUTS // 8)
            tp_gemm(P, sb, ps, [(w_ec3, H2)], [(zcz0_full, H2)], OUTS // 8,
                    act_epilogue(P, sb, zf_sl, bt_ec3, AF.Relu))
            zf_full = allgather(P, dr, zf_sl, [OUTS, NP], "zf_full")

            # ============ out ============
            bt_out = load_bias(P, sb, b_out, NL)

            def ep_out(mi, m_off, m_sz, n_off, n_sz, psum):
                t = sb.tile([m_sz, n_sz], F32, name=P.name("epo"), tag="ep", bufs=3)
                nc.vector.tensor_scalar_add(t[:], psum[:], bt_out[0:m_sz, mi:mi + 1])
                nc.sync.dma_start(outT[m_off:m_off + m_sz, n_off:n_off + n_sz], t[:])
            tp_gemm(P, sb, ps, [(w_out, OUTS)], [(zf_full, OUTS)], NL, ep_out)

    nc.compile()
    legalize_matmul_waits(nc)
    return nc


def shard_inputs(x, gx, edge_index, jw1, jb1, jw2, jb2, ec1_w, ec1_b, dr_w, dr_b,
                 g1_w, g1_b, g2_w, g2_b, ec2_w, ec2_b, ec3_w, ec3_b, out_w, out_b):
    f32 = np.float32
    x = np.asarray(x); gx = np.asarray(gx)
    xp = np.zeros((NP, DX), f32); xp[:N] = x
    gxp = np.zeros((NP, DX), f32); gxp[:N] = gx
    xgT = np.concatenate([xp.T, gxp.T], axis=0).copy()

    row, col = np.asarray(edge_index[0]), np.asarray(edge_index[1])
    deg = np.bincount(col, minlength=N).astype(f32) + 1.0
    dinv = (1.0 / np.sqrt(deg)).astype(f32)
    ahT = np.zeros((NP, NP), f32)
    np.add.at(ahT, (row, col), (dinv[row] * dinv[col]).astype(f32))
    ahT[np.arange(N), np.arange(N)] += dinv * dinv

    jw2p = np.zeros((JH, NP), f32); jw2p[:, :N] = jw2
    jb2p = np.zeros((NP,), f32); jb2p[:N] = jb2
    vmask = np.zeros((1, NP), f32); vmask[0, :N] = 1.0

    def cseg(w, c, width):
        return w[:, c * width:(c + 1) * width]

    ins = []
    for c in range(NCORES):
        cs = slice(c * S, (c + 1) * S)
        eyeT = np.zeros((S, NP), f32)
        rr = np.arange(c * S, min((c + 1) * S, N))
        eyeT[rr - c * S, rr] = 1.0
        w_zp = np.concatenate([cseg(dr_w, c, H2 // 8),
                               cseg(ec2_w[:DX], c, H2 // 8),
                               cseg(g1_w, c, H1 // 8)], axis=1)
        w_z1p = np.concatenate([cseg(ec2_w[DX:DX + H1], c, H2 // 8),
                                cseg(g2_w, c, H2 // 8)], axis=1)
        d = dict(
            xgT=xgT,
            xcol=xp[:, c * (DX // 8):(c + 1) * (DX // 8)],
            ahatT=ahT,
            eyeT=eyeT,
            vmask=vmask,
            w_jw1=cseg(jw1, c, JH // 8),
            b_jb1=np.asarray(jb1)[c * (JH // 8):(c + 1) * (JH // 8)].reshape(-1, 1),
            w_jw2=jw2p[:, cs],
            b_jb2=jb2p[cs].reshape(-1, 1),
            w_ec1x=cseg(ec1_w[:DX], c, H0 // 8),
            w_ec1g=cseg(ec1_w[DX:], c, H0 // 8),
            b_ec1=np.asarray(ec1_b)[c * (H0 // 8):(c + 1) * (H0 // 8)].reshape(-1, 1),
            w_zp=w_zp,
            b_dr=np.asarray(dr_b)[c * (H2 // 8):(c + 1) * (H2 // 8)].reshape(-1, 1),
            w_g1gx=cseg(g1_w, c, H1 // 8),
            b_g1=np.asarray(g1_b)[c * (H1 // 8):(c + 1) * (H1 // 8)].reshape(-1, 1),
            w_z1p=w_z1p,
            b_g2=np.asarray(g2_b)[c * (H2 // 8):(c + 1) * (H2 // 8)].reshape(-1, 1),
            w_ec2c=cseg(ec2_w[DX + H1:], c, H2 // 8),
            b_ec2=np.asarray(ec2_b)[c * (H2 // 8):(c + 1) * (H2 // 8)].reshape(-1, 1),
            w_ec3=cseg(ec3_w, c, OUTS // 8),
            b_ec3=np.asarray(ec3_b)[c * (OUTS // 8):(c + 1) * (OUTS // 8)].reshape(-1, 1),
            w_out=out_w,
            b_out=np.asarray(out_b).reshape(-1, 1),
            identR=np.eye(128, dtype=f32),
            onescol=np.ones((128, 1), f32),
            onesrow=np.ones((1, 128), f32),
        )
        ins.append({k: np.ascontiguousarray(v, dtype=f32) for k, v in d.items()})
    return ins


_PROG = [None]


def kernel(**inputs) -> np.ndarray:
    in_maps = shard_inputs(**inputs)
    if _PROG[0] is None:
        _PROG[0] = build_program()
    nc = _PROG[0]
    res = bass_utils.run_bass_kernel_spmd(nc, in_maps, core_ids=list(range(NCORES)))
    outT = res.results[0]["outT"]
    return np.ascontiguousarray(outT[:, :N].T)
